# revision 34
# baseline (speedup 1.0000x reference)
"""Trainium2 Bass kernel for nn_Attention_4363686773373.

Sigmoid attention with magnitude-preserving (weight-normalized) projections.

Sharding: data-parallel over (batch, T-half) -> 8 shards on 8 NeuronCores.
Each core computes q for its 1024 tokens and k,v for the full 2048 tokens of
its batch. xkv columns are pre-ordered so its query tokens come first.

v2 design (vs the phase-serial baseline): ACT (ScalarE) runs ONLY the
sigmoid wave (192 x [128,1024] from the PSUM score ring) plus one table
load; every rsqrt in the kernel runs on the DVE via the quake bit-trick
seed (int32 shift/sub/xor on a bitcast view) + one RECIPROCAL_APPROX_NR
custom-DVE Newton step (~0.2% worst case, inside the bf16 noise floor).
That removes all sigmoid<->rsqrt ACT table thrash, so k-projections for
later kv-chunks stream through the loop while ACT stays saturated from
~20us onward:

  - w row norms land per-partition ([128, nblocks] PSUM) via K=128/N=1
    matmuls (lhsT = wsq block, rhs = ones column), eliminating the
    baseline's [1,E] row-reduce + DRAM orientation bounce entirely.
  - q/k token norms: per-job [2,tn] ones-reduces are col-tile-packed 4-up
    into one [98,tn] PSUM tile (auto tile_position (0,32j), concurrent on
    PE), one DVE-Newton per packed tile, then a blockdiag expander matmul
    (lhsT at partitions 32j) broadcasts; the k expander carries 8.0
    entries to fold sqrt(HD).
  - fill: q group 0 + k0 group 0 pipelined as the per-dt x/w DMA slices
    land (x split across sync+vector rings, w q-block then k-block on
    scalar, v-block + out_w on gpsimd); first sigmoid ~20us.  Remaining
    fill work (q groups 1-2, k0 tail, v0, k1) drips between chunk-0
    units at an accelerated pace.
  - loop: per chunk, 48 units (score pair -> sigmoid -> attn@v pair,
    both PE-concurrent via row/col tile packing) with v_c and k_{c+1}
    job-stages dripped between units; steady-state PSUM = score ring
    2x[128,1024] (4 banks) + attn@v/v-proj pool (2) + kq stage1 (1) +
    reduce/expand (1) = 8 banks exactly.
  - tail chunk runs th-major; out_w norms, the |x| magnitude chain and
    the whole C-phase for the first t-half (normalize, magnitude
    rescale, out-projection, y DMA) drip between its units; only the
    second t-half's C chain runs after the last sigmoid.  avnT reuses
    qnT's storage (disjoint column lifetimes).  y is written bf16.
"""

import math
from contextlib import ExitStack

import numpy as np

import concourse.bass as bass
import concourse.tile as tile
from concourse import bacc, mybir
from concourse.bass_utils import run_bass_kernel_spmd
from concourse.dve_ops import RECIPROCAL_APPROX_NR

# Problem shapes (hardcoded per harness contract)
B, T, D, H = 4, 2048, 768, 12
HD = D // H  # 64
EPS = 1e-4
SIGMOID_GAIN = 1.8402
N_CORES = 8

F32 = mybir.dt.float32
BF16 = mybir.dt.bfloat16
I32 = mybir.dt.int32
AF = mybir.ActivationFunctionType
ALU = mybir.AluOpType

RSQRT_MAGIC_P1 = 0x5F3759E0  # 0x5f3759df + 1 (for the ~x two's-complement form)


def _ensure_axon_hooks():
    """This image's antenv lacks axon_hooks; reconstruct it so trace=True
    (NTFF profiling) works instead of crashing on import."""
    try:
        import antenv.axon_hooks  # noqa: F401
        return
    except ImportError:
        pass
    import sys
    import types
    try:
        import antenv
    except ImportError:
        return
    mod = types.ModuleType("antenv.axon_hooks")
    _hook = [None]
    mod.set_axon_ntff_profile_hook = lambda h: _hook.__setitem__(0, h)
    mod.get_axon_ntff_profile_hook = lambda: _hook[0]
    sys.modules["antenv.axon_hooks"] = mod
    antenv.axon_hooks = mod
    try:
        from trn_agent_boot.trn_boot import _ntff_profile_via_ctypes
        mod.set_axon_ntff_profile_hook(
            _ntff_profile_via_ctypes('/opt/axon/libaxon_pjrt.so'))
    except Exception:
        pass


_ensure_axon_hooks()


def _chunks(total, maxn=512):
    out = []
    c0 = 0
    while c0 < total:
        cn = min(maxn, total - c0)
        out.append((c0, cn))
        c0 += cn
    return out


def build_program(nc, tc, ctx, Tq, Tkv, Dm, Hn):
    keep = []  # keep tc.tile free-closures alive

    def _tile(shape, dtype, name):
        t, free = tc.tile(shape, dtype, name=name)
        keep.append(free)
        return t

    tc._ant_keepalive = keep
    P = 128
    HDl = 64
    DT = Dm // P              # x/dt slices; also #128-blocks of q/k/v/ow each
    PAIRS = Hn // 2
    E3 = 3 * Dm
    assert DT == PAIRS and PAIRS * P == Dm and Hn * HDl == Dm
    TCH = min(512, Tkv)       # kv token chunk
    NCH = Tkv // TCH
    SBC = TCH // P            # 128-blocks per chunk
    THW = min(512, Tq)        # query tile width
    TH = Tq // THW
    TBkv = Tkv // P
    CW = 2 * P + 64 + 128     # expanders | red32 | one32 | magpat

    xt = nc.dram_tensor("xt", [Dm, Tkv], BF16, kind="ExternalInput").ap()
    wt = nc.dram_tensor("wt", [Dm, E3], BF16, kind="ExternalInput").ap()
    owt = nc.dram_tensor("owt", [Dm, Dm], BF16, kind="ExternalInput").ap()
    cst = nc.dram_tensor("cst", [P, CW], BF16, kind="ExternalInput").ap()
    yt = nc.dram_tensor("yt", [Dm, Tq], BF16, kind="ExternalOutput").ap()
    dbg = getattr(build_program, "_debug", False)
    if dbg:
        d_rwcol = nc.dram_tensor("d_rwcol", [P, 4 * DT], F32,
                                 kind="ExternalOutput").ap()
        d_rw2 = nc.dram_tensor("d_rw2", [P, 3 * DT], F32,
                               kind="ExternalOutput").ap()
        d_knT = nc.dram_tensor("d_knT", [P, PAIRS * Tkv], F32,
                               kind="ExternalOutput").ap()
        d_qnT = nc.dram_tensor("d_qnT", [P, PAIRS * Tq], F32,
                               kind="ExternalOutput").ap()
        d_vbig = nc.dram_tensor("d_vbig", [P, TBkv * Dm], F32,
                                kind="ExternalOutput").ap()
        d_avacc = nc.dram_tensor("d_avacc", [P, PAIRS * Tq], F32,
                                 kind="ExternalOutput").ap()
        d_mag = nc.dram_tensor("d_mag", [1, Tq], F32,
                               kind="ExternalOutput").ap()
        d_mag2 = nc.dram_tensor("d_mag2", [P, Tq], F32,
                                kind="ExternalOutput").ap()
        d_avnT = nc.dram_tensor("d_avnT", [P, PAIRS * Tq], F32,
                                kind="ExternalOutput").ap()

    # ---------------- persistent SBUF ----------------
    xts = _tile([P, DT * Tkv], BF16, "xts")
    wts = _tile([P, DT * E3], BF16, "wts")
    owts = _tile([P, DT * Dm], BF16, "owts")
    knT = _tile([P, PAIRS * Tkv], BF16, "knT")
    qnT = _tile([P, PAIRS * Tq], BF16, "qnT")
    vbig = _tile([P, TBkv * Dm], BF16, "vbig")
    avacc = _tile([P, PAIRS * Tq], F32, "avacc")
    avnT = qnT  # storage reuse: qnT th-columns are dead once avnT(th) written
    rwcol = _tile([P, 4 * DT], F32, "rwcol")    # q,k,v,ow block norms
    rw2col = _tile([P, 3 * DT], F32, "rw2col")  # squared, for q/k/v
    magb16 = _tile([1, Tq], BF16, "magb16")
    mag2 = _tile([P, Tq], BF16, "mag2")         # packed-broadcast magnitude
    csts = _tile([P, CW], BF16, "csts")

    # const APs
    def expQ(j):
        return csts[32 * j:32 * j + 2, 0:P]

    def expK(j):
        return csts[32 * j:32 * j + 2, P:2 * P]

    red32 = csts[:, 2 * P:2 * P + 32]        # [128,32]: col0/1 head masks
    one32 = csts[:, 2 * P + 32:2 * P + 64]   # [128,32]: col0 ones, rest 0
    onescol = csts[:, 2 * P + 32:2 * P + 33]
    magpat = csts[0:1, 2 * P + 64:2 * P + 64 + P]  # [1,128] bcast row

    # ---------------- input DMAs ----------------
    # wire priority across 3 rings: sync=x cols asc; gpsimd=[wq, wv];
    # scalar=[cst, wk, ow]
    nc.scalar.dma_start(csts, cst)
    # tiny dummy sigmoid right after the cst load: pulls the single ACT
    # table load to t~0 (the ACT stream has nothing else before the loop)
    nc.scalar.activation(rwcol[0:1, 0:2], csts[0:1, 0:2], AF.Sigmoid)
    for (c0, cn) in _chunks(Tkv):  # earliest token columns first
        for dt in range(DT):
            nc.sync.dma_start(xts[:, dt * Tkv + c0: dt * Tkv + c0 + cn],
                              xt[dt * P:(dt + 1) * P, c0:c0 + cn])
    for dt in range(DT):  # q weight block on gpsimd ring
        nc.gpsimd.dma_start(wts[:, dt * E3: dt * E3 + Dm],
                            wt[dt * P:(dt + 1) * P, 0:Dm])
    for dt in range(DT):  # k weight block on scalar ring
        nc.scalar.dma_start(wts[:, dt * E3 + Dm: dt * E3 + 2 * Dm],
                            wt[dt * P:(dt + 1) * P, Dm:2 * Dm])
    for dt in range(DT):  # v weight block on gpsimd
        nc.gpsimd.dma_start(wts[:, dt * E3 + 2 * Dm: dt * E3 + 3 * Dm],
                            wt[dt * P:(dt + 1) * P, 2 * Dm:3 * Dm])
    for dt in range(DT):  # out_w last on scalar
        nc.scalar.dma_start(owts[:, dt * Dm:(dt + 1) * Dm],
                            owt[dt * P:(dt + 1) * P, :])

    # ---------------- DVE newton-rsqrt ----------------
    nstk = ExitStack()
    nscr = nstk.enter_context(tc.tile_pool(name="nscr", bufs=2))

    def newton_rsqrt(dst, src, tag):
        """dst = 1/sqrt(src) elementwise.  src: f32 PSUM/SBUF AP; dst may be
        bf16.  Quake seed computed on integer VALUES in fp32 (the DVE does
        arithmetic ops in fp32 regardless of operand dtype, so
        M - bits(src)/2 is exact to ~2^-24, plenty for a 3%% seed), cast
        back to a bit pattern, + one RECIPROCAL_APPROX_NR Newton step:
        y*(1.5 - 0.5*src*y^2)."""
        p, n = src.shape[0], src.shape[1]
        y0i = nscr.tile([p, n], I32, name="nw_y", tag="nwy")
        nc.vector.tensor_scalar(y0i, src.bitcast(I32), -0.5,
                                float(RSQRT_MAGIC_P1 - 1),
                                op0=ALU.mult, op1=ALU.add)
        y0 = y0i.bitcast(F32)
        u = nscr.tile([p, n], F32, name="nw_u", tag="nws")
        nc.vector.scalar_tensor_tensor(u, src, 0.5, y0,
                                       op0=ALU.mult, op1=ALU.mult)
        nc.vector._custom_dve(RECIPROCAL_APPROX_NR, out=dst, in0=u, in1=y0,
                              s0=1.5)

    # persistent job pools (used by fill AND by dripped k-jobs in the loop)
    jstk = ExitStack()
    kst = jstk.enter_context(tc.tile_pool(name="kst", bufs=5))
    nk2p = jstk.enter_context(tc.tile_pool(name="nk2p", bufs=2))

    # ---------------- fill pools ----------------
    fstk = ExitStack()
    wsqp = fstk.enter_context(tc.tile_pool(name="wsqp", bufs=2 * DT))
    nrmp = fstk.enter_context(tc.tile_pool(name="nrmp", bufs=1, space="PSUM"))
    fpk = fstk.enter_context(tc.tile_pool(name="fpk", bufs=5, space="PSUM"))
    fpr = fstk.enter_context(tc.tile_pool(name="fpr", bufs=2, space="PSUM"))

    # ---------------- w row norms (q,k now; v,ow in tail) ----------------
    nrm = nrmp.tile([P, 2 * DT], F32, name="nrm")

    wsq_by_sec = {}

    def wnorm_sq(sec):
        wsqs = []
        for dt in range(DT):
            wsq = wsqp.tile([P, Dm], BF16, name="wsq", tag="wsq")
            ws = wts[:, dt * E3 + sec * Dm: dt * E3 + sec * Dm + Dm]
            nc.vector.tensor_mul(wsq, ws, ws)
            wsqs.append(wsq)
        wsq_by_sec[sec] = wsqs

    def wnorm_fin(sec):
        # PSUM accumulate-flags clear at bank granularity on start=True, so
        # each column group must run dt-contiguously (no interleaving).
        wsqs = wsq_by_sec.pop(sec)
        for j in range(DT):
            for dt in range(DT):
                nc.tensor.matmul(nrm[:, sec * DT + j: sec * DT + j + 1],
                                 lhsT=wsqs[dt][:, j * P:(j + 1) * P],
                                 rhs=onescol,
                                 start=(dt == 0), stop=(dt == DT - 1),
                                 skip_group_check=True)
        newton_rsqrt(rwcol[:, sec * DT:(sec + 1) * DT],
                     nrm[:, sec * DT:(sec + 1) * DT], tag="wn")
        nc.vector.tensor_mul(rw2col[:, sec * DT:(sec + 1) * DT],
                             rwcol[:, sec * DT:(sec + 1) * DT],
                             rwcol[:, sec * DT:(sec + 1) * DT])

    wnorm_sq(0)
    wnorm_sq(1)

    # ---------------- q/k projection job machinery ----------------
    def s1_mm(job, pk_pool, half):
        pr, t0, tn, sec = job["pr"], job["t0"], job["tn"], job["sec"]
        h = (DT + 1) // 2
        rng = range(0, h) if half == 0 else range(h, DT)
        if half == 0:
            job["ps"] = pk_pool.tile([P, tn], F32, name="psK", tag="ps1")
        ps = job["ps"]
        for dt in rng:
            nc.tensor.matmul(
                ps,
                lhsT=wts[:, dt * E3 + sec * Dm + pr * P:
                         dt * E3 + sec * Dm + (pr + 1) * P],
                rhs=xts[:, dt * Tkv + t0: dt * Tkv + t0 + tn],
                start=(dt == 0), stop=(dt == DT - 1))

    def s1_ev(job):
        pr, tn, sec = job["pr"], job["tn"], job["sec"]
        wc = sec * DT + pr
        ps = job.pop("ps")
        kraw = kst.tile([P, tn], BF16, name="kraw", tag="kraw")
        nc.vector.tensor_copy(kraw, ps)
        ksqw = kst.tile([P, tn], BF16, name="ksqw", tag="ksqw")
        nc.vector.scalar_tensor_tensor(ksqw, kraw, rw2col[:, wc:wc + 1], ps,
                                       op0=ALU.mult, op1=ALU.mult)
        job["kraw"], job["ksqw"] = kraw, ksqw

    def s1_emit(job, pk_pool):
        s1_mm(job, pk_pool, 0)
        s1_mm(job, pk_pool, 1)
        s1_ev(job)

    def s2_emit(group, pr_pool):
        g = len(group)
        tn = group[0]["tn"]
        red = pr_pool.tile([32 * g, tn], F32, name="psred", tag="psr")
        for idx, job in enumerate(group):
            nc.tensor.matmul(red[32 * idx:32 * idx + 32, :], lhsT=red32,
                             rhs=job.pop("ksqw"), start=True, stop=True,
                             tile_position=(0, 32 * idx))
        nk2 = nk2p.tile([32 * g, tn], BF16, name="nk2", tag="nk2%d" % g)
        newton_rsqrt(nk2, red[:, :], tag="kq")
        for idx, job in enumerate(group):
            job["nk2"] = (nk2, idx)

    def s3_emit(job, pr_pool):
        pr, t0, tn, sec = job["pr"], job["t0"], job["tn"], job["sec"]
        wc = sec * DT + pr
        nk2, idx = job.pop("nk2")
        psx = pr_pool.tile([P, tn], F32, name="psx", tag="psr")
        nc.tensor.matmul(psx, lhsT=(expQ(idx) if sec == 0 else expK(idx)),
                         rhs=nk2[32 * idx:32 * idx + 2, :],
                         start=True, stop=True, tile_position=(32 * idx, 0))
        dstT, Tdst = (qnT, Tq) if sec == 0 else (knT, Tkv)
        nc.vector.scalar_tensor_tensor(
            dstT[:, pr * Tdst + t0: pr * Tdst + t0 + tn],
            job.pop("kraw"), rwcol[:, wc:wc + 1], psx,
            op0=ALU.mult, op1=ALU.mult)

    def group_tasks(group, pk_pool, pr_pool):
        ts = []
        for j in group:
            ts.append(lambda j=j: s1_mm(j, pk_pool, 0))
            ts.append(lambda j=j: (s1_mm(j, pk_pool, 1), s1_ev(j)))
        ts.append(lambda: s2_emit(group, pr_pool))
        ts += [(lambda j=j: s3_emit(j, pr_pool)) for j in group]
        return ts

    qjobs_th = {th: [dict(pr=pr, t0=th * THW, tn=THW, sec=0)
                     for pr in range(PAIRS)] for th in range(TH)}
    kjobs = {c: [dict(pr=pr, t0=c * TCH, tn=TCH, sec=1)
                 for pr in range(PAIRS)] for c in range(NCH)}

    def grouped(jobs):
        return [jobs[i:i + 4] for i in range(0, len(jobs), 4)]

    # -------- fill: singles-first q/k chains (lowest latency to the first
    # scores), 2-job groups after, all th0 + k0 in the fill --------
    qs0 = qjobs_th[0]
    ks0 = kjobs[0]
    gq = [[qs0[0]]] + ([[qs0[1]]] if len(qs0) > 1 else []) + \
         [qs0[i:i + 2] for i in range(2, len(qs0), 2)]
    gk = [[ks0[0]]] + ([[ks0[1]]] if len(ks0) > 1 else []) + \
         [ks0[i:i + 2] for i in range(2, len(ks0), 2)]
    seq = []
    for i in range(max(len(gq), len(gk))):
        if i == 0:
            seq.append(('q', gq[0], 0))
        if i < len(gk):
            seq.append(('k', gk[i], 1 if i == 0 else None))
        if i + 1 < len(gq):
            seq.append(('q', gq[i + 1], None))
    prev = None
    for kind, g, fin in seq:
        if fin is not None:
            wnorm_fin(fin)
        for j in g:
            s1_emit(j, fpk)
        if prev is not None:
            s2_emit(prev, fpr)
            for j in prev:
                s3_emit(j, fpr)
        prev = g
    s2_emit(prev, fpr)
    for j in prev:
        s3_emit(j, fpr)
    fstk.close()

    # ---------------- loop pools ----------------
    lp = ExitStack()
    psS = lp.enter_context(tc.tile_pool(name="psS", bufs=2, space="PSUM"))
    ps1 = lp.enter_context(tc.tile_pool(name="ps1", bufs=2, space="PSUM"))
    pskp = lp.enter_context(tc.tile_pool(name="pskp", bufs=1, space="PSUM"))
    psrp = lp.enter_context(tc.tile_pool(name="psrp", bufs=1, space="PSUM"))
    attnp = lp.enter_context(tc.tile_pool(name="attnp", bufs=4))
    wsqp2 = lp.enter_context(tc.tile_pool(name="wsqp2", bufs=DT))
    csq = lp.enter_context(tc.tile_pool(name="csq", bufs=4))
    xqp = lp.enter_context(tc.tile_pool(name="xqp", bufs=2))
    csm = lp.enter_context(tc.tile_pool(name="csm", bufs=2))
    yp = lp.enter_context(tc.tile_pool(name="yp", bufs=2))

    def make_vtasks(c):
        tasks = []

        st = {}

        def vtask(tb, e0, en, half):
            def f():
                h = (DT + 1) // 2
                rng = range(0, h) if half == 0 else range(h, DT)
                if half == 0:
                    st[(tb, e0)] = ps1.tile([P, en], F32, name="psV",
                                            tag="ps")
                ps = st[(tb, e0)]
                for dt in rng:
                    nc.tensor.matmul(
                        ps,
                        lhsT=xts[:, dt * Tkv + c * TCH + tb * P:
                                 dt * Tkv + c * TCH + (tb + 1) * P],
                        rhs=wts[:, dt * E3 + 2 * Dm + e0:
                                dt * E3 + 2 * Dm + e0 + en],
                        start=(dt == 0), stop=(dt == DT - 1))
                if half == 1:
                    nc.vector.tensor_copy(
                        vbig[:, (c * SBC + tb) * Dm + e0:
                             (c * SBC + tb) * Dm + e0 + en],
                        st.pop((tb, e0)))
            return f

        for (e0, en) in _chunks(Dm):
            for tb in range(SBC):
                tasks.append(vtask(tb, e0, en, 0))
                tasks.append(vtask(tb, e0, en, 1))
        return tasks

    # ------------- v/ow norms + |x| magnitude (dripped in tail) -----------
    def wn_tail_tasks(sec):
        # sec: 2=v (from wts), 3=ow (from owts); fills rwcol[:, sec*DT:].
        # Bank-granular accumulate-flags: squares first, then dt-contiguous
        # column groups.
        ts = []
        holder = {"sq": []}

        def osq_task(dt):
            def f():
                osq = wsqp2.tile([P, Dm], BF16, name="osq", tag="w2")
                if sec == 2:
                    ws = wts[:, dt * E3 + 2 * Dm: dt * E3 + 3 * Dm]
                else:
                    ws = owts[:, dt * Dm:(dt + 1) * Dm]
                nc.gpsimd.tensor_mul(osq, ws, ws)
                holder["sq"].append(osq)
            return f

        def col_task(j):
            def f():
                if "nrm" not in holder:
                    holder["nrm"] = psrp.tile([P, DT], F32, name="twnrm",
                                              tag="psr")
                for dt in range(DT):
                    nc.tensor.matmul(holder["nrm"][:, j:j + 1],
                                     lhsT=holder["sq"][dt][:, j * P:(j + 1) * P],
                                     rhs=onescol,
                                     start=(dt == 0), stop=(dt == DT - 1),
                                     skip_group_check=True)
            return f

        def fin():
            newton_rsqrt(rwcol[:, sec * DT:(sec + 1) * DT],
                         holder["nrm"][:, :], tag="t%d" % sec)
            if sec == 2:
                nc.vector.tensor_mul(rw2col[:, 2 * DT:3 * DT],
                                     rwcol[:, 2 * DT:3 * DT],
                                     rwcol[:, 2 * DT:3 * DT])

        for dt in range(DT):
            ts.append(osq_task(dt))
        for j in range(DT):
            ts.append(col_task(j))
        ts.append(fin)
        return ts

    def mag_tasks():
        ts = []
        st = {}
        tch = _chunks(Tq)

        def xsq_task(dt):
            def f():
                if "ps0" not in st:
                    # one accumulating slot tile per t-chunk, in different
                    # pools so the groups live in different PSUM banks
                    for i in range(len(tch)):
                        st["ps%d" % i] = (pskp if i % 2 == 0 else psrp).tile(
                            [32, THW], F32, name="psmag",
                            tag="ps1" if i % 2 == 0 else "psr")
                xsq = xqp.tile([P, Tq], BF16, name="xsq", tag="xq")
                xs = xts[:, dt * Tkv: dt * Tkv + Tq]
                nc.gpsimd.tensor_mul(xsq, xs, xs)
                for i, (c0, cn) in enumerate(tch):
                    nc.tensor.matmul(st["ps%d" % i][:, :],
                                     lhsT=one32, rhs=xsq[:, c0:c0 + cn],
                                     start=(dt == 0), stop=(dt == DT - 1),
                                     skip_group_check=True)
            return f

        def mag_fin():
            for i, (c0, cn) in enumerate(tch):
                ps = st["ps%d" % i]
                y = csm.tile([32, THW], F32, name="magy", tag="cf")
                newton_rsqrt(y, ps[:, :], tag="mg")
                magf = csm.tile([32, THW], F32, name="magf", tag="cf")
                nc.vector.scalar_tensor_tensor(magf, ps, math.sqrt(HDl / Dm),
                                               y, op0=ALU.mult, op1=ALU.mult)
                nc.vector.tensor_copy(magb16[0:1, c0:c0 + cn],
                                      magf[0:1, :])

        def mag_bcast(c0, cn):
            def f():
                psb = psrp.tile([P, cn], F32, name="psmb", tag="psr")
                nc.tensor.matmul(psb, lhsT=magpat,
                                 rhs=magb16[0:1, c0:c0 + cn],
                                 start=True, stop=True)
                nc.vector.tensor_copy(mag2[:, c0:c0 + cn], psb)
            return f

        for dt in range(DT):
            ts.append(xsq_task(dt))
        ts.append(mag_fin)
        for (c0, cn) in tch:
            ts.append(mag_bcast(c0, cn))
        return ts

    # ---------------- C phase (normalize + magnitude + out-proj) ----------
    def c_tasks(tc0, tcn):
        ts = []
        st = {}
        prg = [list(range(0, min(4, PAIRS)))]
        if PAIRS > 4:
            prg.append(list(range(4, PAIRS)))
        pools = (pskp, psrp)
        ptag = {id(pskp): "ps1", id(psrp): "psr"}

        def sq_task(pr):
            def f():
                # avacc holds the raw (un-v-normed) accumulation; fold the
                # per-partition v row-norm^2 into the square here.
                sq = csq.tile([P, tcn], BF16, name="sqc", tag="sqc")
                av = avacc[:, pr * Tq + tc0: pr * Tq + tc0 + tcn]
                nc.vector.scalar_tensor_tensor(
                    sq, av, rw2col[:, 2 * DT + pr:2 * DT + pr + 1], av,
                    op0=ALU.mult, op1=ALU.mult)
                st[("sq", pr)] = sq
            return f

        def red_task(gi):
            def f():
                rows = 32 * len(prg[gi])
                pool = pools[gi % 2]
                red = pool.tile([rows, tcn], F32, name="psnc",
                                tag=ptag[id(pool)])
                for idx, pr in enumerate(prg[gi]):
                    nc.tensor.matmul(red[32 * idx:32 * idx + 32, :],
                                     lhsT=red32, rhs=st.pop(("sq", pr)),
                                     start=True, stop=True,
                                     tile_position=(0, 32 * idx))
                ne = csm.tile([rows, tcn], F32, name="ne", tag="cf")
                newton_rsqrt(ne, red[:, :], tag="cn")
                rox = csm.tile([rows, tcn], BF16, name="rox", tag="rox")
                nc.vector.tensor_mul(rox, ne, mag2[0:rows, tc0:tc0 + tcn])
                st[("rox", gi)] = rox
            return f

        def exp_task(gi, idx, pr):
            def f():
                rox = st[("rox", gi)]
                pool = pools[(gi + 1) % 2]
                psx = pool.tile([P, tcn], F32, name="psRX",
                                tag=ptag[id(pool)])
                nc.tensor.matmul(psx, lhsT=expQ(idx),
                                 rhs=rox[32 * idx:32 * idx + 2, :],
                                 start=True, stop=True,
                                 tile_position=(32 * idx, 0))
                nc.vector.scalar_tensor_tensor(
                    avnT[:, pr * Tq + tc0: pr * Tq + tc0 + tcn],
                    avacc[:, pr * Tq + tc0: pr * Tq + tc0 + tcn],
                    rwcol[:, 2 * DT + pr:2 * DT + pr + 1], psx,
                    op0=ALU.mult, op1=ALU.mult)
            return f

        def out_task(eb):
            def f():
                pool = pools[eb % 2]
                psy = pool.tile([P, tcn], F32, name="psY",
                                tag=ptag[id(pool)])
                for dt in range(DT):
                    nc.tensor.matmul(
                        psy,
                        lhsT=owts[:, dt * Dm + eb * P: dt * Dm + (eb + 1) * P],
                        rhs=avnT[:, dt * Tq + tc0: dt * Tq + tc0 + tcn],
                        start=(dt == 0), stop=(dt == DT - 1))
                ysb = yp.tile([P, tcn], BF16, name="ysb", tag="ysb")
                nc.vector.tensor_scalar_mul(
                    ysb, psy, rwcol[:, 3 * DT + eb:3 * DT + eb + 1])
                nc.gpsimd.dma_start(yt[eb * P:(eb + 1) * P, tc0:tc0 + tcn],
                                    ysb)
            return f

        for gi in range(len(prg)):
            for pr in prg[gi]:
                ts.append(sq_task(pr))
            ts.append(red_task(gi))
        for gi in range(len(prg)):
            for idx, pr in enumerate(prg[gi]):
                ts.append(exp_task(gi, idx, pr))
        for eb in range(DT):
            ts.append(out_task(eb))
        return ts

    # ---------------- attention loop ----------------
    psav = {}
    pss_by_u = {}

    def units_of_chunk(c, th_major=False):
        if th_major:
            return [(pr, th, c * SBC + sb) for th in range(TH)
                    for pr in range(PAIRS) for sb in range(SBC)]
        return [(pr, th, c * SBC + sb) for pr in range(PAIRS)
                for th in range(TH) for sb in range(SBC)]

    def emit_scores(u):
        pr, th, sb = u
        pss = psS.tile([P, 2 * THW], F32, name="pss", tag="pss")
        pss_by_u[u] = pss
        for a in (0, 1):
            r0 = a * HDl
            nc.tensor.matmul(
                pss[:, a * THW:(a + 1) * THW],
                lhsT=knT[r0:r0 + HDl,
                         pr * Tkv + sb * P: pr * Tkv + (sb + 1) * P],
                rhs=qnT[r0:r0 + HDl,
                        pr * Tq + th * THW: pr * Tq + (th + 1) * THW],
                start=True, stop=True)

    def emit_unit(u, c):
        pr, th, sb = u
        pss = pss_by_u.pop(u)
        attn = attnp.tile([P, 2 * THW], BF16, name="attn", tag="attn")
        nc.scalar.activation(attn, pss, AF.Sigmoid)
        if (pr, th) not in psav:
            psav[(pr, th)] = ps1.tile([P, THW], F32, name="psav", tag="ps")
        pa = psav[(pr, th)]
        for a in (0, 1):
            r0 = a * HDl
            nc.tensor.matmul(
                pa[r0:r0 + HDl, :],
                lhsT=vbig[:, sb * Dm + pr * P + r0:
                          sb * Dm + pr * P + r0 + HDl],
                rhs=attn[:, a * THW:(a + 1) * THW],
                start=(sb % SBC == 0), stop=(sb % SBC == SBC - 1),
                skip_group_check=True)
        if sb % SBC == SBC - 1:
            pa = psav.pop((pr, th))
            dst = avacc[:, pr * Tq + th * THW: pr * Tq + (th + 1) * THW]
            if c == 0:
                nc.vector.tensor_copy(dst, pa)
            else:
                nc.vector.tensor_tensor(dst, pa, dst, op=ALU.add)

    def run_chunk(c, tasks, th_major=False, pace=1.0):
        us = units_of_chunk(c, th_major=th_major)
        for j in range(min(2, len(us))):
            emit_scores(us[j])
        pi = 0
        eff = max(1, int(len(us) * pace))
        for j, u in enumerate(us):
            # 5-task lookahead: the v-proj halves feeding unit j's attn@v
            # must be EMITTED before the attn@v that reads them (16 v half
            # tasks cover the first 4 units' tb blocks)
            while pi < len(tasks) and (pi - 5) * eff <= j * len(tasks):
                tasks[pi]()
                pi += 1
            emit_unit(u, c)
            if j + 2 < len(us):
                emit_scores(us[j + 2])
        while pi < len(tasks):
            tasks[pi]()
            pi += 1

    for c in range(NCH):
        tail = (c == NCH - 1)
        vt = make_vtasks(c)  # 2 half-tasks per (e-chunk, tb); e0 block first
        nv_e0 = 2 * SBC
        pace = 1.0
        if c == 0:
            # th-major chunk 0: th0 q jobs ran in fill; drip feeds, in
            # deadline order: v0(e0), q th0 tail, k0 tail, v0(e1), q th1
            # (split into 2-job groups), then k1.
            pace = 0.85
            drip = list(vt[0:nv_e0])
            drip += vt[nv_e0:]
            for th in range(1, TH):
                jt = qjobs_th[th]
                for g in [jt[i:i + 2] for i in range(0, len(jt), 2)]:
                    drip += group_tasks(g, pskp, psrp)
            if NCH > 1:
                for g in grouped(kjobs[1]):
                    drip += group_tasks(g, pskp, psrp)
        else:
            drip = list(vt)
            if c + 1 < NCH:
                for g in grouped(kjobs[c + 1]):
                    drip += group_tasks(g, pskp, psrp)
        if tail:
            drip += wn_tail_tasks(2) + wn_tail_tasks(3) + mag_tasks()
            if TH > 1:
                drip += c_tasks(0, THW)
        run_chunk(c, drip, th_major=(tail or c == 0), pace=pace)

    if dbg:
        dbp = ExitStack()
        dpool = dbp.enter_context(tc.tile_pool(name="dbp", bufs=2))

        def _dump(dst, srct):
            tmp = dpool.tile(list(srct.shape), F32, name="dtmp", tag="db")
            nc.vector.tensor_copy(tmp, srct)
            nc.sync.dma_start(dst, tmp)

        _dump(d_knT, knT)
        _dump(d_qnT, qnT)
        _dump(d_vbig, vbig)
        _dump(d_avacc, avacc)

    # C for the remaining t-chunks (straight-line)
    for (c0, cn) in _chunks(Tq):
        if TH > 1 and c0 == 0:
            continue
        for t in c_tasks(c0, cn):
            t()

    if dbg:
        _dump(d_rwcol, rwcol)
        _dump(d_rw2, rw2col)
        _dump(d_mag, magb16)
        _dump(d_mag2, mag2)
        _dump(d_avnT, avnT)
        dbp.close()

    lp.close()
    jstk.close()
    nstk.close()


def make_nc(Tq=T // 2, Tkv=T, Dm=D, Hn=H):
    nc = bacc.Bacc("TRN2", target_bir_lowering=False, debug=False,
                   num_devices=N_CORES)
    with ExitStack() as ctx:
        with tile.TileContext(nc) as tc:
            build_program(nc, tc, ctx, Tq, Tkv, Dm, Hn)
    nc.compile()
    return nc


_CACHED_NC = None


def _get_nc():
    global _CACHED_NC
    if _CACHED_NC is None:
        _CACHED_NC = make_nc()
    return _CACHED_NC


def const_np():
    """Host-built constant tile: blockdiag expanders (q=1.0, k=8.0 at 4
    partition offsets), a [128,32] head-mask reducer (cols 0/1), a [128,32]
    ones-reducer (col 0), and the [1,128] magnitude-broadcast row pattern."""
    import ml_dtypes
    bf16 = ml_dtypes.bfloat16
    P, HDl = 128, 64
    CW = 2 * P + 64 + 128
    cstv = np.zeros((P, CW), np.float32)
    for j in range(4):
        cstv[32 * j, 0:HDl] = 1.0
        cstv[32 * j + 1, HDl:P] = 1.0
        cstv[32 * j, P:P + HDl] = 8.0
        cstv[32 * j + 1, P + HDl:2 * P] = 8.0
    cstv[0:HDl, 2 * P] = 1.0         # red32 col 0: head-0 mask
    cstv[HDl:P, 2 * P + 1] = 1.0     # red32 col 1: head-1 mask
    cstv[:, 2 * P + 32] = 1.0        # one32 col 0: all-ones
    for j in range(4):               # mag bcast row pattern [1, 128]
        cstv[0, 2 * P + 64 + 32 * j] = 1.0
        cstv[0, 2 * P + 64 + 32 * j + 1] = 1.0
    return cstv.astype(bf16)


def _prep_inputs(x, qkv_w, out_w):
    import ml_dtypes
    bf16 = ml_dtypes.bfloat16
    Tq = T // 2
    x = np.asarray(x, dtype=np.float32)
    wtv = np.ascontiguousarray(np.asarray(qkv_w, np.float32).T.astype(bf16))
    owtv = np.ascontiguousarray(np.asarray(out_w, np.float32).T.astype(bf16))
    cstv = const_np()
    in_maps = []
    for core in range(N_CORES):
        b, half = core // 2, core % 2
        own = x[b, half * Tq:(half + 1) * Tq]
        other = x[b, (1 - half) * Tq:(2 - half) * Tq]
        xc = np.concatenate([own, other], axis=0)
        xtc = np.ascontiguousarray(xc.T.astype(bf16))
        in_maps.append({"xt": xtc, "wt": wtv, "owt": owtv, "cst": cstv})
    return in_maps


def run(x, qkv_w, out_w, trace=False, trace_cores=None):
    nc = _get_nc()
    in_maps = _prep_inputs(x, qkv_w, out_w)
    res = run_bass_kernel_spmd(nc, in_maps, list(range(N_CORES)),
                               trace=trace, trace_cores=trace_cores)
    Tq = T // 2
    y = np.empty((B, T, D), np.float32)
    for core, r in enumerate(res.results):
        b, half = core // 2, core % 2
        y[b, half * Tq:(half + 1) * Tq] = \
            np.asarray(r["yt"]).astype(np.float32).T
    return y, res


def kernel(x, qkv_w, out_w):
    y, _ = run(x, qkv_w, out_w, trace=False)
    return y


# revision 37
# speedup vs baseline: 1.0266x; 1.0266x over previous
"""Trainium2 Bass kernel for nn_Attention_4363686773373.

Sigmoid attention with magnitude-preserving (weight-normalized) projections.

Sharding: data-parallel over (batch, T-half) -> 8 shards on 8 NeuronCores.
Each core computes q for its 1024 tokens and k,v for the full 2048 tokens of
its batch. xkv columns are pre-ordered so its query tokens come first.

v2 design (vs the phase-serial baseline): ACT (ScalarE) runs ONLY the
sigmoid wave (192 x [128,1024] from the PSUM score ring) plus one table
load; every rsqrt in the kernel runs on the DVE via the quake bit-trick
seed (int32 shift/sub/xor on a bitcast view) + one RECIPROCAL_APPROX_NR
custom-DVE Newton step (~0.2% worst case, inside the bf16 noise floor).
That removes all sigmoid<->rsqrt ACT table thrash, so k-projections for
later kv-chunks stream through the loop while ACT stays saturated from
~20us onward:

  - w row norms land per-partition ([128, nblocks] PSUM) via K=128/N=1
    matmuls (lhsT = wsq block, rhs = ones column), eliminating the
    baseline's [1,E] row-reduce + DRAM orientation bounce entirely.
  - q/k token norms: per-job [2,tn] ones-reduces are col-tile-packed 4-up
    into one [98,tn] PSUM tile (auto tile_position (0,32j), concurrent on
    PE), one DVE-Newton per packed tile, then a blockdiag expander matmul
    (lhsT at partitions 32j) broadcasts; the k expander carries 8.0
    entries to fold sqrt(HD).
  - fill: q group 0 + k0 group 0 pipelined as the per-dt x/w DMA slices
    land (x split across sync+vector rings, w q-block then k-block on
    scalar, v-block + out_w on gpsimd); first sigmoid ~20us.  Remaining
    fill work (q groups 1-2, k0 tail, v0, k1) drips between chunk-0
    units at an accelerated pace.
  - loop: per chunk, 48 units (score pair -> sigmoid -> attn@v pair,
    both PE-concurrent via row/col tile packing) with v_c and k_{c+1}
    job-stages dripped between units; steady-state PSUM = score ring
    2x[128,1024] (4 banks) + attn@v/v-proj pool (2) + kq stage1 (1) +
    reduce/expand (1) = 8 banks exactly.
  - tail chunk runs th-major; out_w norms, the |x| magnitude chain and
    the whole C-phase for the first t-half (normalize, magnitude
    rescale, out-projection, y DMA) drip between its units; only the
    second t-half's C chain runs after the last sigmoid.  avnT reuses
    qnT's storage (disjoint column lifetimes).  y is written bf16.
"""

import math
from contextlib import ExitStack

import numpy as np

import concourse.bass as bass
import concourse.tile as tile
from concourse import bacc, mybir
from concourse.bass_utils import run_bass_kernel_spmd
from concourse.dve_ops import RECIPROCAL_APPROX_NR

# Problem shapes (hardcoded per harness contract)
B, T, D, H = 4, 2048, 768, 12
HD = D // H  # 64
EPS = 1e-4
SIGMOID_GAIN = 1.8402
N_CORES = 8

F32 = mybir.dt.float32
BF16 = mybir.dt.bfloat16
I32 = mybir.dt.int32
AF = mybir.ActivationFunctionType
ALU = mybir.AluOpType

RSQRT_MAGIC_P1 = 0x5F3759E0  # 0x5f3759df + 1 (for the ~x two's-complement form)


def _ensure_axon_hooks():
    """This image's antenv lacks axon_hooks; reconstruct it so trace=True
    (NTFF profiling) works instead of crashing on import."""
    try:
        import antenv.axon_hooks  # noqa: F401
        return
    except ImportError:
        pass
    import sys
    import types
    try:
        import antenv
    except ImportError:
        return
    mod = types.ModuleType("antenv.axon_hooks")
    _hook = [None]
    mod.set_axon_ntff_profile_hook = lambda h: _hook.__setitem__(0, h)
    mod.get_axon_ntff_profile_hook = lambda: _hook[0]
    sys.modules["antenv.axon_hooks"] = mod
    antenv.axon_hooks = mod
    try:
        from trn_agent_boot.trn_boot import _ntff_profile_via_ctypes
        mod.set_axon_ntff_profile_hook(
            _ntff_profile_via_ctypes('/opt/axon/libaxon_pjrt.so'))
    except Exception:
        pass


_ensure_axon_hooks()


def _chunks(total, maxn=512):
    out = []
    c0 = 0
    while c0 < total:
        cn = min(maxn, total - c0)
        out.append((c0, cn))
        c0 += cn
    return out


def build_program(nc, tc, ctx, Tq, Tkv, Dm, Hn):
    keep = []  # keep tc.tile free-closures alive

    def _tile(shape, dtype, name):
        t, free = tc.tile(shape, dtype, name=name)
        keep.append(free)
        return t

    tc._ant_keepalive = keep
    P = 128
    HDl = 64
    DT = Dm // P              # x/dt slices; also #128-blocks of q/k/v/ow each
    PAIRS = Hn // 2
    E3 = 3 * Dm
    assert DT == PAIRS and PAIRS * P == Dm and Hn * HDl == Dm
    TCH = min(512, Tkv)       # kv token chunk
    NCH = Tkv // TCH
    SBC = TCH // P            # 128-blocks per chunk
    THW = min(512, Tq)        # query tile width
    TH = Tq // THW
    TBkv = Tkv // P
    CW = 2 * P + 64 + 128     # expanders | red32 | one32 | magpat

    xt = nc.dram_tensor("xt", [Dm, Tkv], BF16, kind="ExternalInput").ap()
    wt = nc.dram_tensor("wt", [Dm, E3], BF16, kind="ExternalInput").ap()
    owt = nc.dram_tensor("owt", [Dm, Dm], BF16, kind="ExternalInput").ap()
    cst = nc.dram_tensor("cst", [P, CW], BF16, kind="ExternalInput").ap()
    yt = nc.dram_tensor("yt", [Dm, Tq], BF16, kind="ExternalOutput").ap()
    dbg = getattr(build_program, "_debug", False)
    if dbg:
        d_rwcol = nc.dram_tensor("d_rwcol", [P, 4 * DT], F32,
                                 kind="ExternalOutput").ap()
        d_rw2 = nc.dram_tensor("d_rw2", [P, 3 * DT], F32,
                               kind="ExternalOutput").ap()
        d_knT = nc.dram_tensor("d_knT", [P, PAIRS * Tkv], F32,
                               kind="ExternalOutput").ap()
        d_qnT = nc.dram_tensor("d_qnT", [P, PAIRS * Tq], F32,
                               kind="ExternalOutput").ap()
        d_vbig = nc.dram_tensor("d_vbig", [P, TBkv * Dm], F32,
                                kind="ExternalOutput").ap()
        d_avacc = nc.dram_tensor("d_avacc", [P, PAIRS * Tq], F32,
                                 kind="ExternalOutput").ap()
        d_mag = nc.dram_tensor("d_mag", [1, Tq], F32,
                               kind="ExternalOutput").ap()
        d_mag2 = nc.dram_tensor("d_mag2", [P, Tq], F32,
                                kind="ExternalOutput").ap()
        d_avnT = nc.dram_tensor("d_avnT", [P, PAIRS * Tq], F32,
                                kind="ExternalOutput").ap()

    # ---------------- persistent SBUF ----------------
    xts = _tile([P, DT * Tkv], BF16, "xts")
    wts = _tile([P, DT * E3], BF16, "wts")
    owts = _tile([P, DT * Dm], BF16, "owts")
    knT = _tile([P, PAIRS * Tkv], BF16, "knT")
    qnT = _tile([P, PAIRS * Tq], BF16, "qnT")
    vbig = _tile([P, TBkv * Dm], BF16, "vbig")
    avacc = _tile([P, PAIRS * Tq], F32, "avacc")
    avnT = qnT  # storage reuse: qnT th-columns are dead once avnT(th) written
    rwcol = _tile([P, 4 * DT], F32, "rwcol")    # q,k,v,ow block norms
    rw2col = _tile([P, 3 * DT], F32, "rw2col")  # squared, for q/k/v
    magb16 = _tile([1, Tq], BF16, "magb16")
    mag2 = _tile([P, Tq], BF16, "mag2")         # packed-broadcast magnitude
    csts = _tile([P, CW], BF16, "csts")

    # const APs
    def expQ(j):
        return csts[32 * j:32 * j + 2, 0:P]

    def expK(j):
        return csts[32 * j:32 * j + 2, P:2 * P]

    red32 = csts[:, 2 * P:2 * P + 32]        # [128,32]: col0/1 head masks
    one32 = csts[:, 2 * P + 32:2 * P + 64]   # [128,32]: col0 ones, rest 0
    onescol = csts[:, 2 * P + 32:2 * P + 33]
    magpat = csts[0:1, 2 * P + 64:2 * P + 64 + P]  # [1,128] bcast row

    # ---------------- input DMAs ----------------
    # wire-priority order: [wq || wk] -> [x cols asc || wv] -> ow
    nc.scalar.dma_start(csts, cst)
    # tiny dummy sigmoid right after the cst load: pulls the single ACT
    # table load to t~0 (the ACT stream has nothing else before the loop)
    nc.scalar.activation(rwcol[0:1, 0:2], csts[0:1, 0:2], AF.Sigmoid)
    for dt in range(DT):  # q weight block first on sync
        nc.sync.dma_start(wts[:, dt * E3: dt * E3 + Dm],
                          wt[dt * P:(dt + 1) * P, 0:Dm])
    for (c0, cn) in _chunks(Tkv):  # earliest token columns first
        for dt in range(DT):
            nc.sync.dma_start(xts[:, dt * Tkv + c0: dt * Tkv + c0 + cn],
                              xt[dt * P:(dt + 1) * P, c0:c0 + cn])
    for dt in range(DT):  # k weight block early on scalar
        nc.scalar.dma_start(wts[:, dt * E3 + Dm: dt * E3 + 2 * Dm],
                            wt[dt * P:(dt + 1) * P, Dm:2 * Dm])
    for dt in range(DT):
        nc.scalar.dma_start(wts[:, dt * E3 + 2 * Dm: dt * E3 + 3 * Dm],
                            wt[dt * P:(dt + 1) * P, 2 * Dm:3 * Dm])
    for dt in range(DT):
        nc.scalar.dma_start(owts[:, dt * Dm:(dt + 1) * Dm],
                            owt[dt * P:(dt + 1) * P, :])

    # ---------------- DVE newton-rsqrt ----------------
    nstk = ExitStack()
    nscr = nstk.enter_context(tc.tile_pool(name="nscr", bufs=2))

    def newton_rsqrt(dst, src, tag):
        """dst = 1/sqrt(src) elementwise.  src: f32 PSUM/SBUF AP; dst may be
        bf16.  Quake seed computed on integer VALUES in fp32 (the DVE does
        arithmetic ops in fp32 regardless of operand dtype, so
        M - bits(src)/2 is exact to ~2^-24, plenty for a 3%% seed), cast
        back to a bit pattern, + one RECIPROCAL_APPROX_NR Newton step:
        y*(1.5 - 0.5*src*y^2)."""
        p, n = src.shape[0], src.shape[1]
        y0i = nscr.tile([p, n], I32, name="nw_y", tag="nwy")
        nc.vector.tensor_scalar(y0i, src.bitcast(I32), -0.5,
                                float(RSQRT_MAGIC_P1 - 1),
                                op0=ALU.mult, op1=ALU.add)
        y0 = y0i.bitcast(F32)
        u = nscr.tile([p, n], F32, name="nw_u", tag="nws")
        nc.vector.scalar_tensor_tensor(u, src, 0.5, y0,
                                       op0=ALU.mult, op1=ALU.mult)
        nc.vector._custom_dve(RECIPROCAL_APPROX_NR, out=dst, in0=u, in1=y0,
                              s0=1.5)

    # persistent job pools (used by fill AND by dripped k-jobs in the loop)
    jstk = ExitStack()
    kst = jstk.enter_context(tc.tile_pool(name="kst", bufs=5))
    nk2p = jstk.enter_context(tc.tile_pool(name="nk2p", bufs=2))

    # ---------------- fill pools ----------------
    fstk = ExitStack()
    wsqp = fstk.enter_context(tc.tile_pool(name="wsqp", bufs=DT))
    nrmp = fstk.enter_context(tc.tile_pool(name="nrmp", bufs=1, space="PSUM"))
    fpk = fstk.enter_context(tc.tile_pool(name="fpk", bufs=4, space="PSUM"))
    fpr = fstk.enter_context(tc.tile_pool(name="fpr", bufs=2, space="PSUM"))

    # ---------------- w row norms (q,k now; v,ow in tail) ----------------
    nrm = nrmp.tile([P, 2 * DT], F32, name="nrm")

    def wnorm_sec(sec):
        # sec: 0=q, 1=k -> w cols [sec*Dm,(sec+1)*Dm), rwcol cols sec*DT+.
        # PSUM accumulate-flags clear at bank granularity on start=True, so
        # each column group must run dt-contiguously (no interleaving).
        wsqs = []
        for dt in range(DT):
            wsq = wsqp.tile([P, Dm], BF16, name="wsq", tag="wsq")
            ws = wts[:, dt * E3 + sec * Dm: dt * E3 + sec * Dm + Dm]
            nc.vector.tensor_mul(wsq, ws, ws)
            wsqs.append(wsq)
        for j in range(DT):
            for dt in range(DT):
                nc.tensor.matmul(nrm[:, sec * DT + j: sec * DT + j + 1],
                                 lhsT=wsqs[dt][:, j * P:(j + 1) * P],
                                 rhs=onescol,
                                 start=(dt == 0), stop=(dt == DT - 1),
                                 skip_group_check=True)
        newton_rsqrt(rwcol[:, sec * DT:(sec + 1) * DT],
                     nrm[:, sec * DT:(sec + 1) * DT], tag="wn")
        nc.vector.tensor_mul(rw2col[:, sec * DT:(sec + 1) * DT],
                             rwcol[:, sec * DT:(sec + 1) * DT],
                             rwcol[:, sec * DT:(sec + 1) * DT])

    wnorm_sec(0)

    # ---------------- q/k projection job machinery ----------------
    def s1_mm(job, pk_pool, half):
        pr, t0, tn, sec = job["pr"], job["t0"], job["tn"], job["sec"]
        h = (DT + 1) // 2
        rng = range(0, h) if half == 0 else range(h, DT)
        if half == 0:
            job["ps"] = pk_pool.tile([P, tn], F32, name="psK", tag="ps1")
        ps = job["ps"]
        for dt in rng:
            nc.tensor.matmul(
                ps,
                lhsT=wts[:, dt * E3 + sec * Dm + pr * P:
                         dt * E3 + sec * Dm + (pr + 1) * P],
                rhs=xts[:, dt * Tkv + t0: dt * Tkv + t0 + tn],
                start=(dt == 0), stop=(dt == DT - 1))

    def s1_ev(job):
        pr, tn, sec = job["pr"], job["tn"], job["sec"]
        wc = sec * DT + pr
        ps = job.pop("ps")
        kraw = kst.tile([P, tn], BF16, name="kraw", tag="kraw")
        nc.vector.tensor_copy(kraw, ps)
        ksqw = kst.tile([P, tn], BF16, name="ksqw", tag="ksqw")
        nc.vector.scalar_tensor_tensor(ksqw, kraw, rw2col[:, wc:wc + 1], ps,
                                       op0=ALU.mult, op1=ALU.mult)
        job["kraw"], job["ksqw"] = kraw, ksqw

    def s1_emit(job, pk_pool):
        s1_mm(job, pk_pool, 0)
        s1_mm(job, pk_pool, 1)
        s1_ev(job)

    def s2_emit(group, pr_pool):
        g = len(group)
        tn = group[0]["tn"]
        red = pr_pool.tile([32 * g, tn], F32, name="psred", tag="psr")
        for idx, job in enumerate(group):
            nc.tensor.matmul(red[32 * idx:32 * idx + 32, :], lhsT=red32,
                             rhs=job.pop("ksqw"), start=True, stop=True,
                             tile_position=(0, 32 * idx))
        nk2 = nk2p.tile([32 * g, tn], BF16, name="nk2", tag="nk2%d" % g)
        newton_rsqrt(nk2, red[:, :], tag="kq")
        for idx, job in enumerate(group):
            job["nk2"] = (nk2, idx)

    def s3_emit(job, pr_pool):
        pr, t0, tn, sec = job["pr"], job["t0"], job["tn"], job["sec"]
        wc = sec * DT + pr
        nk2, idx = job.pop("nk2")
        psx = pr_pool.tile([P, tn], F32, name="psx", tag="psr")
        nc.tensor.matmul(psx, lhsT=(expQ(idx) if sec == 0 else expK(idx)),
                         rhs=nk2[32 * idx:32 * idx + 2, :],
                         start=True, stop=True, tile_position=(32 * idx, 0))
        dstT, Tdst = (qnT, Tq) if sec == 0 else (knT, Tkv)
        nc.vector.scalar_tensor_tensor(
            dstT[:, pr * Tdst + t0: pr * Tdst + t0 + tn],
            job.pop("kraw"), rwcol[:, wc:wc + 1], psx,
            op0=ALU.mult, op1=ALU.mult)

    def group_tasks(group, pk_pool, pr_pool):
        ts = []
        for j in group:
            ts.append(lambda j=j: s1_mm(j, pk_pool, 0))
            ts.append(lambda j=j: (s1_mm(j, pk_pool, 1), s1_ev(j)))
        ts.append(lambda: s2_emit(group, pr_pool))
        ts += [(lambda j=j: s3_emit(j, pr_pool)) for j in group]
        return ts

    qjobs_th = {th: [dict(pr=pr, t0=th * THW, tn=THW, sec=0)
                     for pr in range(PAIRS)] for th in range(TH)}
    kjobs = {c: [dict(pr=pr, t0=c * TCH, tn=TCH, sec=1)
                 for pr in range(PAIRS)] for c in range(NCH)}

    def grouped(jobs):
        return [jobs[i:i + 4] for i in range(0, len(jobs), 4)]

    # -------- fill: w-norms for q and k (squares on DVE, pipelined with
    # the weight DMAs), then the th0 q group-A chain and the k0 group-A
    # chain software-pipelined --------
    wnorm_sec(1)
    qgA = grouped(qjobs_th[0])
    kg0 = grouped(kjobs[0])
    for j in qgA[0]:
        s1_emit(j, fpk)
    s2_emit(qgA[0], fpr)
    for j in kg0[0]:
        s1_emit(j, fpk)
    for j in qgA[0]:
        s3_emit(j, fpr)
    s2_emit(kg0[0], fpr)
    for j in kg0[0]:
        s3_emit(j, fpr)
    fstk.close()

    # ---------------- loop pools ----------------
    lp = ExitStack()
    psS = lp.enter_context(tc.tile_pool(name="psS", bufs=2, space="PSUM"))
    ps1 = lp.enter_context(tc.tile_pool(name="ps1", bufs=2, space="PSUM"))
    pskp = lp.enter_context(tc.tile_pool(name="pskp", bufs=1, space="PSUM"))
    psrp = lp.enter_context(tc.tile_pool(name="psrp", bufs=1, space="PSUM"))
    attnp = lp.enter_context(tc.tile_pool(name="attnp", bufs=4))
    wsqp2 = lp.enter_context(tc.tile_pool(name="wsqp2", bufs=DT))
    csq = lp.enter_context(tc.tile_pool(name="csq", bufs=4))
    xqp = lp.enter_context(tc.tile_pool(name="xqp", bufs=2))
    csm = lp.enter_context(tc.tile_pool(name="csm", bufs=2))
    yp = lp.enter_context(tc.tile_pool(name="yp", bufs=2))

    def make_vtasks(c):
        tasks = []

        st = {}

        def vtask(tb, e0, en, half):
            def f():
                h = (DT + 1) // 2
                rng = range(0, h) if half == 0 else range(h, DT)
                if half == 0:
                    st[(tb, e0)] = ps1.tile([P, en], F32, name="psV",
                                            tag="ps")
                ps = st[(tb, e0)]
                for dt in rng:
                    nc.tensor.matmul(
                        ps,
                        lhsT=xts[:, dt * Tkv + c * TCH + tb * P:
                                 dt * Tkv + c * TCH + (tb + 1) * P],
                        rhs=wts[:, dt * E3 + 2 * Dm + e0:
                                dt * E3 + 2 * Dm + e0 + en],
                        start=(dt == 0), stop=(dt == DT - 1))
                if half == 1:
                    nc.vector.tensor_copy(
                        vbig[:, (c * SBC + tb) * Dm + e0:
                             (c * SBC + tb) * Dm + e0 + en],
                        st.pop((tb, e0)))
            return f

        for (e0, en) in _chunks(Dm):
            for tb in range(SBC):
                tasks.append(vtask(tb, e0, en, 0))
                tasks.append(vtask(tb, e0, en, 1))
        return tasks

    # ------------- v/ow norms + |x| magnitude (dripped in tail) -----------
    def wn_tail_tasks(sec):
        # sec: 2=v (from wts), 3=ow (from owts); fills rwcol[:, sec*DT:].
        # Bank-granular accumulate-flags: squares first, then dt-contiguous
        # column groups.
        ts = []
        holder = {"sq": []}

        def osq_task(dt):
            def f():
                osq = wsqp2.tile([P, Dm], BF16, name="osq", tag="w2")
                if sec == 2:
                    ws = wts[:, dt * E3 + 2 * Dm: dt * E3 + 3 * Dm]
                else:
                    ws = owts[:, dt * Dm:(dt + 1) * Dm]
                nc.gpsimd.tensor_mul(osq, ws, ws)
                holder["sq"].append(osq)
            return f

        def col_task(j):
            def f():
                if "nrm" not in holder:
                    holder["nrm"] = psrp.tile([P, DT], F32, name="twnrm",
                                              tag="psr")
                for dt in range(DT):
                    nc.tensor.matmul(holder["nrm"][:, j:j + 1],
                                     lhsT=holder["sq"][dt][:, j * P:(j + 1) * P],
                                     rhs=onescol,
                                     start=(dt == 0), stop=(dt == DT - 1),
                                     skip_group_check=True)
            return f

        def fin():
            newton_rsqrt(rwcol[:, sec * DT:(sec + 1) * DT],
                         holder["nrm"][:, :], tag="t%d" % sec)
            if sec == 2:
                nc.vector.tensor_mul(rw2col[:, 2 * DT:3 * DT],
                                     rwcol[:, 2 * DT:3 * DT],
                                     rwcol[:, 2 * DT:3 * DT])

        for dt in range(DT):
            ts.append(osq_task(dt))
        for j in range(DT):
            ts.append(col_task(j))
        ts.append(fin)
        return ts

    def mag_tasks():
        ts = []
        st = {}
        tch = _chunks(Tq)

        def xsq_task(dt):
            def f():
                if "ps0" not in st:
                    # one accumulating slot tile per t-chunk, in different
                    # pools so the groups live in different PSUM banks
                    for i in range(len(tch)):
                        st["ps%d" % i] = (pskp if i % 2 == 0 else psrp).tile(
                            [32, THW], F32, name="psmag",
                            tag="ps1" if i % 2 == 0 else "psr")
                xsq = xqp.tile([P, Tq], BF16, name="xsq", tag="xq")
                xs = xts[:, dt * Tkv: dt * Tkv + Tq]
                nc.gpsimd.tensor_mul(xsq, xs, xs)
                for i, (c0, cn) in enumerate(tch):
                    nc.tensor.matmul(st["ps%d" % i][:, :],
                                     lhsT=one32, rhs=xsq[:, c0:c0 + cn],
                                     start=(dt == 0), stop=(dt == DT - 1),
                                     skip_group_check=True)
            return f

        def mag_fin():
            for i, (c0, cn) in enumerate(tch):
                ps = st["ps%d" % i]
                y = csm.tile([32, THW], F32, name="magy", tag="cf")
                newton_rsqrt(y, ps[:, :], tag="mg")
                magf = csm.tile([32, THW], F32, name="magf", tag="cf")
                nc.vector.scalar_tensor_tensor(magf, ps, math.sqrt(HDl / Dm),
                                               y, op0=ALU.mult, op1=ALU.mult)
                nc.vector.tensor_copy(magb16[0:1, c0:c0 + cn],
                                      magf[0:1, :])

        def mag_bcast(c0, cn):
            def f():
                psb = psrp.tile([P, cn], F32, name="psmb", tag="psr")
                nc.tensor.matmul(psb, lhsT=magpat,
                                 rhs=magb16[0:1, c0:c0 + cn],
                                 start=True, stop=True)
                nc.vector.tensor_copy(mag2[:, c0:c0 + cn], psb)
            return f

        for dt in range(DT):
            ts.append(xsq_task(dt))
        ts.append(mag_fin)
        for (c0, cn) in tch:
            ts.append(mag_bcast(c0, cn))
        return ts

    # ---------------- C phase (normalize + magnitude + out-proj) ----------
    def c_tasks(tc0, tcn):
        ts = []
        st = {}
        prg = [list(range(0, min(4, PAIRS)))]
        if PAIRS > 4:
            prg.append(list(range(4, PAIRS)))
        pools = (pskp, psrp)
        ptag = {id(pskp): "ps1", id(psrp): "psr"}

        def sq_task(pr):
            def f():
                # avacc holds the raw (un-v-normed) accumulation; fold the
                # per-partition v row-norm^2 into the square here.
                sq = csq.tile([P, tcn], BF16, name="sqc", tag="sqc")
                av = avacc[:, pr * Tq + tc0: pr * Tq + tc0 + tcn]
                nc.vector.scalar_tensor_tensor(
                    sq, av, rw2col[:, 2 * DT + pr:2 * DT + pr + 1], av,
                    op0=ALU.mult, op1=ALU.mult)
                st[("sq", pr)] = sq
            return f

        def red_task(gi):
            def f():
                rows = 32 * len(prg[gi])
                pool = pools[gi % 2]
                red = pool.tile([rows, tcn], F32, name="psnc",
                                tag=ptag[id(pool)])
                for idx, pr in enumerate(prg[gi]):
                    nc.tensor.matmul(red[32 * idx:32 * idx + 32, :],
                                     lhsT=red32, rhs=st.pop(("sq", pr)),
                                     start=True, stop=True,
                                     tile_position=(0, 32 * idx))
                ne = csm.tile([rows, tcn], F32, name="ne", tag="cf")
                newton_rsqrt(ne, red[:, :], tag="cn")
                rox = csm.tile([rows, tcn], BF16, name="rox", tag="rox")
                nc.vector.tensor_mul(rox, ne, mag2[0:rows, tc0:tc0 + tcn])
                st[("rox", gi)] = rox
            return f

        def exp_task(gi, idx, pr):
            def f():
                rox = st[("rox", gi)]
                pool = pools[(gi + 1) % 2]
                psx = pool.tile([P, tcn], F32, name="psRX",
                                tag=ptag[id(pool)])
                nc.tensor.matmul(psx, lhsT=expQ(idx),
                                 rhs=rox[32 * idx:32 * idx + 2, :],
                                 start=True, stop=True,
                                 tile_position=(32 * idx, 0))
                nc.vector.scalar_tensor_tensor(
                    avnT[:, pr * Tq + tc0: pr * Tq + tc0 + tcn],
                    avacc[:, pr * Tq + tc0: pr * Tq + tc0 + tcn],
                    rwcol[:, 2 * DT + pr:2 * DT + pr + 1], psx,
                    op0=ALU.mult, op1=ALU.mult)
            return f

        def out_task(eb, half):
            def f():
                pool = pools[eb % 2]
                h = (DT + 1) // 2
                rng = range(0, h) if half == 0 else range(h, DT)
                if half == 0:
                    st[("psy", eb)] = pool.tile([P, tcn], F32, name="psY",
                                                tag=ptag[id(pool)])
                psy = st[("psy", eb)]
                for dt in rng:
                    nc.tensor.matmul(
                        psy,
                        lhsT=owts[:, dt * Dm + eb * P: dt * Dm + (eb + 1) * P],
                        rhs=avnT[:, dt * Tq + tc0: dt * Tq + tc0 + tcn],
                        start=(dt == 0), stop=(dt == DT - 1))
                if half == 1:
                    psy = st.pop(("psy", eb))
                    ysb = yp.tile([P, tcn], BF16, name="ysb", tag="ysb")
                    nc.vector.tensor_scalar_mul(
                        ysb, psy, rwcol[:, 3 * DT + eb:3 * DT + eb + 1])
                    nc.gpsimd.dma_start(
                        yt[eb * P:(eb + 1) * P, tc0:tc0 + tcn], ysb)
            return f

        for gi in range(len(prg)):
            for pr in prg[gi]:
                ts.append(sq_task(pr))
            ts.append(red_task(gi))
        for gi in range(len(prg)):
            for idx, pr in enumerate(prg[gi]):
                ts.append(exp_task(gi, idx, pr))
        for eb in range(DT):
            ts.append(out_task(eb, 0))
            ts.append(out_task(eb, 1))
        return ts

    # ---------------- attention loop ----------------
    psav = {}
    pss_by_u = {}

    def units_of_chunk(c, th_major=False):
        if th_major:
            return [(pr, th, c * SBC + sb) for th in range(TH)
                    for pr in range(PAIRS) for sb in range(SBC)]
        return [(pr, th, c * SBC + sb) for pr in range(PAIRS)
                for th in range(TH) for sb in range(SBC)]

    def emit_scores(u):
        pr, th, sb = u
        pss = psS.tile([P, 2 * THW], F32, name="pss", tag="pss")
        pss_by_u[u] = pss
        for a in (0, 1):
            r0 = a * HDl
            nc.tensor.matmul(
                pss[:, a * THW:(a + 1) * THW],
                lhsT=knT[r0:r0 + HDl,
                         pr * Tkv + sb * P: pr * Tkv + (sb + 1) * P],
                rhs=qnT[r0:r0 + HDl,
                        pr * Tq + th * THW: pr * Tq + (th + 1) * THW],
                start=True, stop=True)

    def emit_unit(u, c):
        pr, th, sb = u
        pss = pss_by_u.pop(u)
        attn = attnp.tile([P, 2 * THW], BF16, name="attn", tag="attn")
        nc.scalar.activation(attn, pss, AF.Sigmoid)
        if (pr, th) not in psav:
            psav[(pr, th)] = ps1.tile([P, THW], F32, name="psav", tag="ps")
        pa = psav[(pr, th)]
        for a in (0, 1):
            r0 = a * HDl
            nc.tensor.matmul(
                pa[r0:r0 + HDl, :],
                lhsT=vbig[:, sb * Dm + pr * P + r0:
                          sb * Dm + pr * P + r0 + HDl],
                rhs=attn[:, a * THW:(a + 1) * THW],
                start=(sb % SBC == 0), stop=(sb % SBC == SBC - 1),
                skip_group_check=True)
        if sb % SBC == SBC - 1:
            pa = psav.pop((pr, th))
            dst = avacc[:, pr * Tq + th * THW: pr * Tq + (th + 1) * THW]
            if c == 0:
                nc.vector.tensor_copy(dst, pa)
            else:
                nc.vector.tensor_tensor(dst, pa, dst, op=ALU.add)

    def run_chunk(c, tasks, th_major=False, pace=1.0):
        us = units_of_chunk(c, th_major=th_major)
        for j in range(min(2, len(us))):
            emit_scores(us[j])
        pi = 0
        eff = max(1, int(len(us) * pace))
        for j, u in enumerate(us):
            # 5-task lookahead: the v-proj halves feeding unit j's attn@v
            # must be EMITTED before the attn@v that reads them (16 v half
            # tasks cover the first 4 units' tb blocks)
            while pi < len(tasks) and (pi - 5) * eff <= j * len(tasks):
                tasks[pi]()
                pi += 1
            emit_unit(u, c)
            if j + 2 < len(us):
                emit_scores(us[j + 2])
        return tasks[pi:]  # leftovers carry into the next chunk's drip

    carry = []
    c_rest = []
    for c in range(NCH):
        tail = (c == NCH - 1)
        vt = make_vtasks(c)  # 2 half-tasks per (e-chunk, tb); e0 block first
        nv_e0 = 2 * SBC
        pace = 1.0
        if c == 0:
            # th-major chunk 0: th0 q jobs ran in fill; drip feeds, in
            # deadline order: v0(e0), q th0 tail, k0 tail, v0(e1), q th1
            # (split into 2-job groups), then k1.
            pace = 0.85
            drip = list(vt[0:nv_e0])
            for g in qgA[1:]:
                drip += group_tasks(g, pskp, psrp)
            for g in kg0[1:]:
                drip += group_tasks(g, pskp, psrp)
            drip += vt[nv_e0:]
            for th in range(1, TH):
                jt = qjobs_th[th]
                for g in [jt[i:i + 2] for i in range(0, len(jt), 2)]:
                    drip += group_tasks(g, pskp, psrp)
            if NCH > 1:
                for g in grouped(kjobs[1]):
                    drip += group_tasks(g, pskp, psrp)
        else:
            drip = list(vt)
            if c + 1 < NCH:
                for g in grouped(kjobs[c + 1]):
                    drip += group_tasks(g, pskp, psrp)
        if tail:
            drip += wn_tail_tasks(2) + wn_tail_tasks(3) + mag_tasks()
            if TH > 1:
                drip += c_tasks(0, THW)
                # second t-half C: the pr0-3 normalize chain (folds land by
                # ~unit 39 of the th-major tail) drips at the tail's end;
                # the rest runs post-loop
                ct1 = c_tasks(THW, Tq - THW)
                nhead = min(4, PAIRS) + 1  # sq(g0) + red/newton g0
                drip += ct1[:nhead]
                c_rest = ct1[nhead:]
        carry = run_chunk(c, carry + drip, th_major=(tail or c == 0),
                          pace=pace)

    if dbg:
        dbp = ExitStack()
        dpool = dbp.enter_context(tc.tile_pool(name="dbp", bufs=2))

        def _dump(dst, srct):
            tmp = dpool.tile(list(srct.shape), F32, name="dtmp", tag="db")
            nc.vector.tensor_copy(tmp, srct)
            nc.sync.dma_start(dst, tmp)

        _dump(d_knT, knT)
        _dump(d_qnT, qnT)
        _dump(d_vbig, vbig)
        _dump(d_avacc, avacc)

    for t in carry:  # tail leftovers
        t()
    # C for the remaining t-chunks (straight-line)
    if TH > 1:
        for t in c_rest:
            t()
    else:
        for (c0, cn) in _chunks(Tq):
            for t in c_tasks(c0, cn):
                t()

    if dbg:
        _dump(d_rwcol, rwcol)
        _dump(d_rw2, rw2col)
        _dump(d_mag, magb16)
        _dump(d_mag2, mag2)
        _dump(d_avnT, avnT)
        dbp.close()

    lp.close()
    jstk.close()
    nstk.close()


def make_nc(Tq=T // 2, Tkv=T, Dm=D, Hn=H):
    nc = bacc.Bacc("TRN2", target_bir_lowering=False, debug=False,
                   num_devices=N_CORES)
    with ExitStack() as ctx:
        with tile.TileContext(nc) as tc:
            build_program(nc, tc, ctx, Tq, Tkv, Dm, Hn)
    nc.compile()
    return nc


_CACHED_NC = None


def _get_nc():
    global _CACHED_NC
    if _CACHED_NC is None:
        _CACHED_NC = make_nc()
    return _CACHED_NC


def const_np():
    """Host-built constant tile: blockdiag expanders (q=1.0, k=8.0 at 4
    partition offsets), a [128,32] head-mask reducer (cols 0/1), a [128,32]
    ones-reducer (col 0), and the [1,128] magnitude-broadcast row pattern."""
    import ml_dtypes
    bf16 = ml_dtypes.bfloat16
    P, HDl = 128, 64
    CW = 2 * P + 64 + 128
    cstv = np.zeros((P, CW), np.float32)
    for j in range(4):
        cstv[32 * j, 0:HDl] = 1.0
        cstv[32 * j + 1, HDl:P] = 1.0
        cstv[32 * j, P:P + HDl] = 8.0
        cstv[32 * j + 1, P + HDl:2 * P] = 8.0
    cstv[0:HDl, 2 * P] = 1.0         # red32 col 0: head-0 mask
    cstv[HDl:P, 2 * P + 1] = 1.0     # red32 col 1: head-1 mask
    cstv[:, 2 * P + 32] = 1.0        # one32 col 0: all-ones
    for j in range(4):               # mag bcast row pattern [1, 128]
        cstv[0, 2 * P + 64 + 32 * j] = 1.0
        cstv[0, 2 * P + 64 + 32 * j + 1] = 1.0
    return cstv.astype(bf16)


def _prep_inputs(x, qkv_w, out_w):
    import ml_dtypes
    bf16 = ml_dtypes.bfloat16
    Tq = T // 2
    x = np.asarray(x, dtype=np.float32)
    wtv = np.ascontiguousarray(np.asarray(qkv_w, np.float32).T.astype(bf16))
    owtv = np.ascontiguousarray(np.asarray(out_w, np.float32).T.astype(bf16))
    cstv = const_np()
    in_maps = []
    for core in range(N_CORES):
        b, half = core // 2, core % 2
        own = x[b, half * Tq:(half + 1) * Tq]
        other = x[b, (1 - half) * Tq:(2 - half) * Tq]
        xc = np.concatenate([own, other], axis=0)
        xtc = np.ascontiguousarray(xc.T.astype(bf16))
        in_maps.append({"xt": xtc, "wt": wtv, "owt": owtv, "cst": cstv})
    return in_maps


def run(x, qkv_w, out_w, trace=False, trace_cores=None):
    nc = _get_nc()
    in_maps = _prep_inputs(x, qkv_w, out_w)
    res = run_bass_kernel_spmd(nc, in_maps, list(range(N_CORES)),
                               trace=trace, trace_cores=trace_cores)
    Tq = T // 2
    y = np.empty((B, T, D), np.float32)
    for core, r in enumerate(res.results):
        b, half = core // 2, core % 2
        y[b, half * Tq:(half + 1) * Tq] = \
            np.asarray(r["yt"]).astype(np.float32).T
    return y, res


def kernel(x, qkv_w, out_w):
    y, _ = run(x, qkv_w, out_w, trace=False)
    return y
